# revision 1
# baseline (speedup 1.0000x reference)
"""GCN (3x GCNConv + global max pool + FC + log_softmax) on 8 Trainium2 NeuronCores.

Strategy:
  - 1D partition of nodes: core c owns rows [12500c, 12500(c+1)).
  - Per conv layer l: each core computes hs = dinv * (a @ W_l) for its slice
    (PE matmul, bf16), AllGather -> full table tbl_l [100000, 128] bf16 in DRAM.
  - Aggregation: per-edge gather of hs rows via gpsimd dma_gather (int16
    indices => 4 source chunks of 25000 rows), then segment-sum via PE matmuls
    with per-128-edge one-hot selection matrices built on DVE (is_equal of
    dst-in-tile index vs an iota row), accumulated in PSUM per 128-dst tile.
  - out = relu(dinv * agg + b); transposed on PE to feed the next dense matmul.
  - Pooling (segment max over graphs), tiny FC and log_softmax run on host
    from the returned conv3 node features (0.01% of FLOPs).
"""

import sys

sys.path.insert(0, "/opt/trn_rl_repo")

import numpy as np
import ml_dtypes

import concourse.bass as bass
import concourse.bacc as bacc
import concourse.tile as tile
from concourse import mybir
from concourse.bass_utils import run_bass_kernel_spmd
from concourse.masks import make_identity

P = 128
N_NODES = 100000
N_EDGES = 1600000
N_GRAPHS = 64
N_CORES = 8
NODES_PER_CORE = N_NODES // N_CORES          # 12500
NTILES = (NODES_PER_CORE + P - 1) // P       # 98 (last tile 84 rows)
LAST_ROWS = NODES_PER_CORE - (NTILES - 1) * P  # 84
NCHUNK = 4
CHUNK = N_NODES // NCHUNK                    # 25000
WT = 8                                       # tiles per wave
F0 = 512
FW = 128                                     # table width (all convs padded to 128)
MAX_CALL_BLOCKS = 47                         # ~6K idxs/call (tested); single_packet=False required >1024
NQUEUES = 4
DMA_SCRATCH = 16384
dt = mybir.dt
BF = ml_dtypes.bfloat16


def _rows(t):
    return LAST_ROWS if t == NTILES - 1 else P


def _host_prep(edge_index):
    """Build the shared (cross-core) aggregation schedule + per-core index data."""
    src = np.concatenate([edge_index[0], np.arange(N_NODES, dtype=np.int64)]).astype(np.int64)
    dst = np.concatenate([edge_index[1], np.arange(N_NODES, dtype=np.int64)]).astype(np.int64)
    deg = np.bincount(dst, minlength=N_NODES).astype(np.float32)
    dinv = (1.0 / np.sqrt(deg)).astype(np.float32)

    waves = [list(range(w, min(w + WT, NTILES))) for w in range(0, NTILES, WT)]

    # per-core grouped edges
    core_of = dst // NODES_PER_CORE
    ch_of = src // CHUNK
    per_core = []
    cnts = np.zeros((N_CORES, NTILES, NCHUNK), np.int64)
    for c in range(N_CORES):
        m = core_of == c
        s, d = src[m], dst[m]
        dl = d - c * NODES_PER_CORE
        t = dl // P
        key = t * NCHUNK + ch_of[m]
        o = np.argsort(key, kind="stable")
        s, dl, t_, key = s[o], dl[o], t[o], key[o]
        cnt = np.bincount(key, minlength=NTILES * NCHUNK).reshape(NTILES, NCHUNK)
        cnts[c] = cnt
        per_core.append((s, dl, cnt))

    blocks = np.maximum((cnts.max(axis=0) + P - 1) // P, 1)  # [NTILES, NCHUNK] shared

    # slot bookkeeping (shared): per chunk, slots ordered (wave, tile, block)
    S_ch = [int(blocks[:, ch].sum() * P) for ch in range(NCHUNK)]
    NB_total = int(blocks.sum())

    # didx column order: (wave, tile, chunk, block)
    # per-(w,ch) msg-tile col offset for tile t: cumsum of blocks over earlier tiles in wave
    idx16 = [np.zeros((N_CORES, 128, S_ch[ch] // 16), np.int16) for ch in range(NCHUNK)]
    didx = np.full((N_CORES, 128, NB_total), -1.0, np.float32)

    # precompute per-chunk slot starts for (w, t)
    chunk_start = [dict() for _ in range(NCHUNK)]
    for ch in range(NCHUNK):
        pos = 0
        for w, wtiles in enumerate(waves):
            for t in wtiles:
                chunk_start[ch][t] = pos
                pos += int(blocks[t, ch]) * P
    # didx col starts per (t, ch)
    g_start = {}
    g = 0
    for w, wtiles in enumerate(waves):
        for t in wtiles:
            for ch in range(NCHUNK):
                g_start[(t, ch)] = g
                g += int(blocks[t, ch])
    assert g == NB_total

    for c in range(N_CORES):
        s, dl, cnt = per_core[c]
        # group boundaries in the (t, ch)-sorted arrays
        ends = np.cumsum(cnt.reshape(-1))
        starts = ends - cnt.reshape(-1)
        idx_slots = [np.zeros(S_ch[ch], np.int16) for ch in range(NCHUNK)]
        didx_slots = np.full(NB_total * P, -1.0, np.float32)
        for t in range(NTILES):
            for ch in range(NCHUNK):
                k = t * NCHUNK + ch
                n = cnt[t, ch]
                if n == 0:
                    continue
                e0, e1 = starts[k], ends[k]
                ss = s[e0:e1] - ch * CHUNK
                dd = dl[e0:e1] - t * P
                cs = chunk_start[ch][t]
                idx_slots[ch][cs:cs + n] = ss.astype(np.int16)
                gs = g_start[(t, ch)] * P
                didx_slots[gs:gs + n] = dd.astype(np.float32)
        for ch in range(NCHUNK):
            w16 = idx_slots[ch].reshape(-1, 16).T  # [16, S/16]
            idx16[ch][c] = np.tile(w16, (8, 1))
        didx[c] = didx_slots.reshape(-1, P).T

    meta = {
        "waves": waves,
        "blocks": blocks,
        "S_ch": S_ch,
        "NB_total": NB_total,
        "chunk_start": chunk_start,
        "g_start": g_start,
    }
    return dinv, idx16, didx, meta


def _build_program(meta):
    waves = meta["waves"]
    blocks = meta["blocks"]
    S_ch = meta["S_ch"]
    NB_total = meta["NB_total"]
    chunk_start = meta["chunk_start"]
    g_start = meta["g_start"]

    nc = bacc.Bacc(
        "TRN2", target_bir_lowering=False, debug=False, num_devices=N_CORES,
        num_swdge_queues=NQUEUES, dynamic_dma_scratch_size=DMA_SCRATCH,
    )

    xT_io = nc.dram_tensor("xT", [F0, NODES_PER_CORE], dt.bfloat16, kind="ExternalInput").ap()
    dinv_io = nc.dram_tensor("dinvT", [P, NTILES], dt.float32, kind="ExternalInput").ap()
    w1_io = nc.dram_tensor("W1sb", [P, F0], dt.bfloat16, kind="ExternalInput").ap()
    w2_io = nc.dram_tensor("W2pad", [P, P], dt.bfloat16, kind="ExternalInput").ap()
    w3_io = nc.dram_tensor("W3pad", [P, P], dt.bfloat16, kind="ExternalInput").ap()
    b1_io = nc.dram_tensor("b1rep", [P, P], dt.float32, kind="ExternalInput").ap()
    b2_io = nc.dram_tensor("b2rep", [P, P], dt.float32, kind="ExternalInput").ap()
    b3_io = nc.dram_tensor("b3rep", [P, 64], dt.float32, kind="ExternalInput").ap()
    iota_io = nc.dram_tensor("iota", [P, P], dt.bfloat16, kind="ExternalInput").ap()
    idx_ios = [
        nc.dram_tensor(f"idx{ch}", [P, S_ch[ch] // 16], dt.int16, kind="ExternalInput").ap()
        for ch in range(NCHUNK)
    ]
    didx_io = nc.dram_tensor("didx", [P, NB_total], dt.bfloat16, kind="ExternalInput").ap()
    out_io = nc.dram_tensor("out3", [NODES_PER_CORE, 64], dt.float32, kind="ExternalOutput").ap()

    with tile.TileContext(nc) as tc:
        with (
            tc.tile_pool(name="const", bufs=1) as constp,
            tc.tile_pool(name="aT", bufs=1) as aTp,
            tc.tile_pool(name="xT", bufs=2) as xTp,
            tc.tile_pool(name="idxw", bufs=8) as idxp,
            tc.tile_pool(name="msgs", bufs=6) as msgp,
            tc.tile_pool(name="sel", bufs=3) as selp,
            tc.tile_pool(name="work", bufs=3) as workp,
            tc.tile_pool(name="hs", bufs=3) as hsp,
            tc.tile_pool(name="psum_d", bufs=2, space="PSUM") as psdp,
            tc.tile_pool(name="psum_a", bufs=4, space="PSUM") as psap,
            tc.tile_pool(name="psum_t", bufs=2, space="PSUM") as pstp,
            tc.tile_pool(name="dram", bufs=1, space="DRAM") as dramp,
        ):
            # ---- constants ----
            iota_t = constp.tile([P, P], dt.bfloat16)
            nc.sync.dma_start(iota_t[:], iota_io[:])
            dinv_sb = constp.tile([P, NTILES], dt.float32)
            nc.sync.dma_start(dinv_sb[:], dinv_io[:])
            w1_sb = constp.tile([P, F0], dt.bfloat16)
            nc.sync.dma_start(w1_sb[:], w1_io[:])
            w2_sb = constp.tile([P, P], dt.bfloat16)
            nc.sync.dma_start(w2_sb[:], w2_io[:])
            w3_sb = constp.tile([P, P], dt.bfloat16)
            nc.sync.dma_start(w3_sb[:], w3_io[:])
            b1_sb = constp.tile([P, P], dt.float32)
            nc.sync.dma_start(b1_sb[:], b1_io[:])
            b2_sb = constp.tile([P, P], dt.float32)
            nc.sync.dma_start(b2_sb[:], b2_io[:])
            b3_sb = constp.tile([P, 64], dt.float32)
            nc.sync.dma_start(b3_sb[:], b3_io[:])
            didx_sb = constp.tile([P, NB_total], dt.bfloat16)
            nc.sync.dma_start(didx_sb[:], didx_io[:])
            ident = constp.tile([P, P], dt.float32)
            make_identity(nc, ident[:])

            a2T = aTp.tile([P, NODES_PER_CORE], dt.bfloat16, tag="a2T")
            a3T = aTp.tile([P, NODES_PER_CORE], dt.bfloat16, tag="a3T")

            tbls = [dramp.tile([N_NODES, FW], dt.bfloat16, tag=f"tbl{l}", name=f"tbl{l}") for l in range(3)]
            bounces = [dramp.tile([NODES_PER_CORE, FW], dt.bfloat16, tag=f"bnc{l}", name=f"bnc{l}") for l in range(3)]

            def dense_phase(l):
                """hs_l = dinv * (a @ W) -> bounce[l]; AllGather -> tbls[l]."""
                for t in range(NTILES):
                    r = _rows(t)
                    c0 = t * P
                    ps = psdp.tile([r, P], dt.float32, space="PSUM", tag="pd")
                    if l == 0:
                        nk = F0 // P
                        for k in range(nk):
                            xt = xTp.tile([P, P], dt.bfloat16, tag="xt")
                            nc.sync.dma_start(xt[:, :r], xT_io[k * P:(k + 1) * P, c0:c0 + r])
                            nc.tensor.matmul(
                                out=ps[:], lhsT=xt[:, :r], rhs=w1_sb[:, k * P:(k + 1) * P],
                                start=(k == 0), stop=(k == nk - 1),
                            )
                    else:
                        aT = a2T if l == 1 else a3T
                        w = w2_sb if l == 1 else w3_sb
                        nc.tensor.matmul(
                            out=ps[:], lhsT=aT[:, c0:c0 + r], rhs=w[:],
                            start=True, stop=True,
                        )
                    hs = hsp.tile([r, P], dt.bfloat16, tag="hs")
                    nc.vector.tensor_scalar_mul(hs[:], ps[:], dinv_sb[:r, t:t + 1])
                    nc.sync.dma_start(bounces[l][c0:c0 + r, :], hs[:])
                nc.gpsimd.collective_compute(
                    "AllGather", mybir.AluOpType.bypass,
                    replica_groups=[list(range(N_CORES))],
                    ins=[bounces[l].opt()], outs=[tbls[l].opt()],
                )

            def agg_phase(l):
                """agg from tbls[l]; out = relu(dinv*agg + b); store a(l+2)T or out3."""
                import os as _os
                agg_mode = _os.environ.get("GCN_AGG_MODE", "full")
                b_sb = (b1_sb, b2_sb, b3_sb)[l]
                ncols = P if l < 2 else 64
                for w, wtiles in enumerate(waves):
                    # gathers (one per chunk)
                    msg_tiles = {}
                    for ch in range(NCHUNK):
                        nb = int(blocks[wtiles, ch].sum()) if False else int(sum(blocks[t, ch] for t in wtiles))
                        s0 = chunk_start[ch][wtiles[0]]
                        S = nb * P
                        iw = idxp.tile([P, S // 16], dt.int16, tag=f"idx")
                        nc.sync.dma_start(iw[:], idx_ios[ch][:, s0 // 16:(s0 + S) // 16])
                        mt = msgp.tile([P, nb, FW], dt.bfloat16, tag="msg")
                        if agg_mode == "nogather":
                            nc.vector.memset(mt[:], 0.0)
                        else:
                            for b0 in range(0, nb, MAX_CALL_BLOCKS):
                                b1 = min(b0 + MAX_CALL_BLOCKS, nb)
                                Ssub = (b1 - b0) * P
                                nc.gpsimd.dma_gather(
                                    out_ap=mt[:, b0:b1, :],
                                    in_ap=tbls[l][ch * CHUNK:(ch + 1) * CHUNK, :],
                                    idxs_ap=iw[:, b0 * P // 16:b1 * P // 16],
                                    num_idxs=Ssub, num_idxs_reg=Ssub,
                                    elem_size=FW, elem_step=FW,
                                    single_packet=False,
                                    queue_num=(b0 // MAX_CALL_BLOCKS + ch) % NQUEUES,
                                )
                        msg_tiles[ch] = mt
                    if agg_mode == "gatheronly":
                        continue

                    # selection matrices, batches of 8 blocks in didx col order
                    gw0 = g_start[(wtiles[0], 0)]
                    gw1 = gw0 + int(sum(blocks[t, ch] for t in wtiles for ch in range(NCHUNK)))
                    BB = 8
                    sel_tiles = {}
                    for q0 in range(gw0, gw1, BB):
                        q1 = min(q0 + BB, gw1)
                        st = selp.tile([P, BB, P], dt.bfloat16, tag="sel")
                        nc.vector.tensor_tensor(
                            out=st[:, :q1 - q0, :],
                            in0=didx_sb[:, q0:q1, None].to_broadcast([P, q1 - q0, P]),
                            in1=iota_t[:, None, :].to_broadcast([P, q1 - q0, P]),
                            op=mybir.AluOpType.is_equal,
                        )
                        for q in range(q0, q1):
                            sel_tiles[q] = (st, q - q0)

                    # per-tile matmul accumulation + post
                    for ti, t in enumerate(wtiles):
                        r = _rows(t)
                        c0 = t * P
                        ps = psap.tile([r, ncols], dt.float32, space="PSUM", tag="pa")
                        mms = []
                        for ch in range(NCHUNK):
                            coff = int(sum(blocks[tt, ch] for tt in wtiles[:ti]))
                            for b in range(int(blocks[t, ch])):
                                mms.append((g_start[(t, ch)] + b, ch, coff + b))
                        for i, (q, ch, col) in enumerate(mms):
                            st, j = sel_tiles[q]
                            nc.tensor.matmul(
                                out=ps[:], lhsT=st[:, j, :r],
                                rhs=msg_tiles[ch][:, col, :ncols],
                                start=(i == 0), stop=(i == len(mms) - 1),
                            )
                        scaled = workp.tile([P, ncols], dt.float32, tag="scaled")
                        nc.vector.tensor_scalar_mul(scaled[:r], ps[:], dinv_sb[:r, t:t + 1])
                        withb = workp.tile([P, ncols], dt.float32, tag="withb")
                        nc.vector.tensor_add(withb[:r], scaled[:r], b_sb[:r, :ncols])
                        outt = workp.tile([P, ncols], dt.float32, tag="outt")
                        nc.scalar.activation(outt[:r], withb[:r], mybir.ActivationFunctionType.Relu)
                        if l < 2:
                            aT = a2T if l == 0 else a3T
                            pst = pstp.tile([P, P], dt.float32, space="PSUM", tag="pt")
                            nc.tensor.transpose(out=pst[:, :r], in_=outt[:r], identity=ident[:r, :r])
                            nc.vector.tensor_copy(aT[:, c0:c0 + r], pst[:, :r])
                        else:
                            nc.sync.dma_start(out_io[c0:c0 + r, :], outt[:r])

            import os
            nphase = int(os.environ.get("GCN_NPHASE", "6"))
            for l in range(3):
                if 2 * l < nphase:
                    dense_phase(l)
                if 2 * l + 1 <= nphase - 1:
                    agg_phase(l)
            if nphase < 5:
                # ensure out3 is written so the output tensor exists
                zz = workp.tile([P, 64], dt.float32, tag="outt")
                nc.gpsimd.memset(zz[:], 0.0)
                for t in range(NTILES):
                    nc.sync.dma_start(out_io[t * P:t * P + _rows(t), :], zz[:_rows(t)])

    nc.compile()
    return nc


def _pack_inputs(x, dinv, W1, b1, W2, b2, W3, b3, idx16, didx):
    iota_rep = np.tile(np.arange(P, dtype=np.float32)[None, :], (P, 1)).astype(BF)
    # W1 packed: W1sb[i, 128k+j] = W1[128k+i, j]
    w1sb = np.zeros((P, F0), np.float32)
    for k in range(F0 // P):
        w1sb[:, k * P:(k + 1) * P] = W1[k * P:(k + 1) * P, :]
    w2pad = np.zeros((P, P), np.float32)
    w2pad[:, :64] = W2
    w3pad = np.zeros((P, P), np.float32)
    w3pad[:64, :32] = W3
    b1rep = np.tile(b1[None, :], (P, 1)).astype(np.float32)
    b2rep = np.zeros((P, P), np.float32)
    b2rep[:, :64] = b2[None, :]
    b3rep = np.zeros((P, 64), np.float32)
    b3rep[:, :32] = b3[None, :]

    in_maps = []
    for c in range(N_CORES):
        lo = c * NODES_PER_CORE
        xs = x[lo:lo + NODES_PER_CORE].astype(np.float32)
        dvt = np.ones((P, NTILES), np.float32)
        dv = dinv[lo:lo + NODES_PER_CORE]
        for t in range(NTILES):
            r = _rows(t)
            dvt[:r, t] = dv[t * P:t * P + r]
        in_maps.append({
            "xT": np.ascontiguousarray(xs.T).astype(BF),
            "dinvT": dvt,
            "W1sb": w1sb.astype(BF),
            "W2pad": w2pad.astype(BF),
            "W3pad": w3pad.astype(BF),
            "b1rep": b1rep, "b2rep": b2rep, "b3rep": b3rep,
            "iota": iota_rep,
            **{f"idx{ch}": idx16[ch][c] for ch in range(NCHUNK)},
            "didx": didx[c].astype(BF),
        })
    return in_maps


_TRACE = [False]          # set by test harness to request a profiled run
_LAST_RESULT = [None]     # BassKernelResults of the last run (for profiling)


def kernel(x, edge_index, batch, W1, b1, W2, b2, W3, b3, Wfc, bfc):
    x = np.asarray(x)
    edge_index = np.asarray(edge_index)
    batch = np.asarray(batch)
    W1, b1 = np.asarray(W1), np.asarray(b1)
    W2, b2 = np.asarray(W2), np.asarray(b2)
    W3, b3 = np.asarray(W3), np.asarray(b3)
    Wfc, bfc = np.asarray(Wfc), np.asarray(bfc)

    dinv, idx16, didx, meta = _host_prep(edge_index.astype(np.int64))
    nc = _build_program(meta)
    in_maps = _pack_inputs(x, dinv, W1, b1, W2, b2, W3, b3, idx16, didx)
    res = run_bass_kernel_spmd(
        nc, in_maps, core_ids=list(range(N_CORES)), trace=_TRACE[0]
    )
    _LAST_RESULT[0] = res

    h3 = np.concatenate([res.results[c]["out3"][:, :32] for c in range(N_CORES)], axis=0)

    # host epilogue: segment max pool + FC + log_softmax (float64 for stability)
    pooled = np.full((N_GRAPHS, 32), -np.inf, np.float64)
    bnd = np.searchsorted(batch, np.arange(N_GRAPHS + 1))
    for g in range(N_GRAPHS):
        if bnd[g + 1] > bnd[g]:
            pooled[g] = h3[bnd[g]:bnd[g + 1]].max(axis=0)
    logits = pooled @ Wfc.astype(np.float64) + bfc.astype(np.float64)
    m = logits.max(axis=1, keepdims=True)
    lse = m + np.log(np.exp(logits - m).sum(axis=1, keepdims=True))
    return (logits - lse).astype(np.float32)



# revision 14
# speedup vs baseline: 2.0819x; 2.0819x over previous
"""GCN (3x GCNConv + global max pool + FC + log_softmax) on 8 Trainium2 NeuronCores.

Strategy (v2, pipelined):
  - 1D partition of nodes: core c owns rows [12500c, 12500(c+1)).
  - Table for layer l holds hs_l = dinv * (a_l @ W_l) rows (bf16, 128-wide,
    256B — the dma_gather minimum elem size), split into 4 "chunks", each the
    AllGather (quarter-interleaved across cores) of one quarter of every
    core's rows.  Quarter AllGathers (Shared outputs) pipeline inside the
    surrounding aggregation work instead of forming a layer barrier.
  - Aggregation per 128-dst tile: per-edge rows gathered via gpsimd
    dma_gather (int16 chunk-local indices), summed on the PE with host-built
    fp8 one-hot selection matrices (shipped as input, reused for all 3
    layers -> no DVE is_equal work, no DVE<->SWDGE SBUF port contention).
  - Self-loops never gathered: identity-matmul from the SBUF-resident local
    hs tile; bias via a K=1 outer-product matmul that also initializes PSUM.
  - Post per tile fused: scalar-engine relu(dinv * psum) -> bf16, PE
    transpose, scalar copy, then the NEXT layer's dense matmul + dinv scale
    immediately (no separate dense phase, no persistent aT buffers).
  - Pooling (segment max over graphs), tiny FC and log_softmax on host.
"""

import os
import sys

sys.path.insert(0, "/opt/trn_rl_repo")

import numpy as np
import ml_dtypes

import concourse.bass as bass
import concourse.bacc as bacc
import concourse.tile as tile
from concourse import mybir
from concourse.bass_utils import run_bass_kernel_spmd

P = 128
N_NODES = 100000
N_EDGES = 1600000
N_GRAPHS = 64
N_CORES = 8
NPC = N_NODES // N_CORES                     # 12500
NTILES = (NPC + P - 1) // P                  # 98 (last tile 84 rows)
LAST_ROWS = NPC - (NTILES - 1) * P           # 84
F0 = 512
FW = 128                                     # table row width (256B gather elem)
NCH = 4                                      # chunks = dst quarters of each core
QT = [0, 25, 50, 75, 98]                     # quarter tile boundaries
QSTART = [0, 3200, 6400, 9600]               # quarter row starts
QROWS = [3200, 3200, 3200, 2900]             # rows per quarter per core
CH_ROWS = [N_CORES * r for r in QROWS]       # table rows per chunk
NCOLS = [128, 64, 32]                        # real table width per layer
WT = int(os.environ.get("GCN_WT", "7"))      # target tiles per wave
MAXB = int(os.environ.get("GCN_MAXB", "17"))  # max blocks per dma_gather call
NQUEUES = 4
DMA_SCRATCH = int(os.environ.get("GCN_SCRATCH", "32768"))
PAD_IDX = int(os.environ.get("GCN_PAD", "0"))  # mid-stream negatives are illegal
SEL_F8 = os.environ.get("GCN_SELDT", "f8") == "f8"
TBL_SHARED = os.environ.get("GCN_SHARED", "1") == "1"
MSG_BUFS = int(os.environ.get("GCN_MSGB", "8"))
SEL_BUFS = int(os.environ.get("GCN_SELB", "2"))
dt = mybir.dt
BF = ml_dtypes.bfloat16
F8 = ml_dtypes.float8_e4m3
SEL_DT = dt.float8e4 if SEL_F8 else dt.bfloat16
SEL_NP = F8 if SEL_F8 else BF


def _rows(t):
    return LAST_ROWS if t == NTILES - 1 else P


def _quarter_of_tile(t):
    for q in range(4):
        if t < QT[q + 1]:
            return q
    raise AssertionError


def _host_prep(edge_index):
    """Shared (cross-core) aggregation schedule + per-core index/sel data."""
    src = edge_index[0].astype(np.int64)
    dst = edge_index[1].astype(np.int64)
    # degrees include self-loops (reference adds them)
    deg = np.bincount(dst, minlength=N_NODES).astype(np.float64) + 1.0
    dinv = (1.0 / np.sqrt(deg)).astype(np.float32)

    # waves: per dst quarter, split its tiles into near-even groups of <= WT
    waves = []  # list of (quarter, [tiles])
    for q in range(4):
        tiles = list(range(QT[q], QT[q + 1]))
        nw = (len(tiles) + WT - 1) // WT
        for part in np.array_split(np.array(tiles), nw):
            waves.append((q, [int(t) for t in part]))
    NW = len(waves)

    # chunk position of a src node: quarter-interleaved table layout
    s_core = src // NPC
    s_loc = src % NPC
    s_tile = s_loc // P
    s_q = np.digitize(s_tile, QT[1:4])              # 0..3
    qrows = np.array(QROWS)[s_q]
    qstart = np.array(QSTART)[s_q]
    s_pos = s_core * qrows + (s_loc - qstart)       # position within chunk s_q

    core_of = dst // NPC
    per_core = []
    cnts = np.zeros((N_CORES, NTILES, NCH), np.int64)
    for c in range(N_CORES):
        m = core_of == c
        pos, ch = s_pos[m], s_q[m]
        dl = dst[m] - c * NPC
        t = dl // P
        key = t * NCH + ch
        o = np.argsort(key, kind="stable")
        pos, ch, dl, t, key = pos[o], ch[o], dl[o], t[o], key[o]
        cnt = np.bincount(key, minlength=NTILES * NCH).reshape(NTILES, NCH)
        cnts[c] = cnt
        per_core.append((pos, dl - t * P, cnt))

    blocks = (cnts.max(axis=0) + P - 1) // P        # [NTILES, NCH] shared

    # global block (sel) order: (wave, tile, chunk, block)
    g_start = {}
    g = 0
    for q, wtiles in waves:
        for t in wtiles:
            for ch in range(NCH):
                g_start[(t, ch)] = g
                g += int(blocks[t, ch])
    NB_total = g

    # idx slot order per chunk: (wave, tile, block)
    chunk_start = [dict() for _ in range(NCH)]
    S_ch = [0] * NCH
    for ch in range(NCH):
        pos = 0
        for q, wtiles in waves:
            for t in wtiles:
                chunk_start[ch][t] = pos
                pos += int(blocks[t, ch]) * P
        S_ch[ch] = pos

    idx16 = [np.zeros((N_CORES, P, S_ch[ch] // 16), np.int16) for ch in range(NCH)]
    sel = np.zeros((N_CORES, P, NB_total, P), SEL_NP)

    for c in range(N_CORES):
        pos_a, dd_a, cnt = per_core[c]
        flat = cnt.reshape(-1)
        ends = np.cumsum(flat)
        starts = ends - flat
        # within-group offsets for every edge
        j = np.arange(len(pos_a)) - np.repeat(starts, flat)
        grp_t = np.repeat(np.arange(NTILES * NCH) // NCH, flat)
        grp_c = np.repeat(np.arange(NTILES * NCH) % NCH, flat)
        # selection entries
        g0 = np.array([g_start[(t, ch)] for t in range(NTILES) for ch in range(NCH)])
        blk = np.repeat(g0, flat) + j // P
        sel[c][j % P, blk, dd_a] = 1.0
        # chunk-local idx slots
        cs0 = np.array(
            [chunk_start[ch][t] for t in range(NTILES) for ch in range(NCH)]
        )
        slot = np.repeat(cs0, flat) + j
        for ch in range(NCH):
            slots = np.full(S_ch[ch], PAD_IDX, np.int16)
            m = grp_c == ch
            slots[slot[m]] = pos_a[m].astype(np.int16)
            w16 = slots.reshape(-1, 16).T
            idx16[ch][c] = np.tile(w16, (8, 1))

    meta = {
        "waves": waves,
        "blocks": blocks,
        "S_ch": S_ch,
        "NB_total": NB_total,
        "chunk_start": chunk_start,
        "g_start": g_start,
    }
    return dinv, idx16, sel, meta


def _build_program(meta):
    waves = meta["waves"]
    blocks = meta["blocks"]
    S_ch = meta["S_ch"]
    NB_total = meta["NB_total"]
    chunk_start = meta["chunk_start"]
    g_start = meta["g_start"]

    # per-wave static shapes
    wave_nb = []     # [wave][ch] blocks
    wave_nbtot = []  # total blocks in wave
    for q, wtiles in waves:
        nbs = [int(sum(blocks[t, ch] for t in wtiles)) for ch in range(NCH)]
        wave_nb.append(nbs)
        wave_nbtot.append(int(sum(nbs)))
    MSG_NB = max(max(nbs) for nbs in wave_nb)
    SEL_NB = max(wave_nbtot)

    nc = bacc.Bacc(
        "TRN2", target_bir_lowering=False, debug=False, num_devices=N_CORES,
        num_swdge_queues=NQUEUES, dynamic_dma_scratch_size=DMA_SCRATCH,
    )

    xT_io = nc.dram_tensor("xT", [F0, NPC], dt.bfloat16, kind="ExternalInput").ap()
    dinv_io = nc.dram_tensor("dinvT", [P, NTILES], dt.float32, kind="ExternalInput").ap()
    rdinv_io = nc.dram_tensor("rdinvR", [1, NTILES * P], dt.bfloat16, kind="ExternalInput").ap()
    w1_io = nc.dram_tensor("W1sb", [P, F0], dt.bfloat16, kind="ExternalInput").ap()
    w2_io = nc.dram_tensor("W2sb", [P, 64], dt.bfloat16, kind="ExternalInput").ap()
    w3_io = nc.dram_tensor("W3sb", [64, 32], dt.bfloat16, kind="ExternalInput").ap()
    brow_ios = [
        nc.dram_tensor(f"b{l+1}row", [1, NCOLS[l]], dt.bfloat16, kind="ExternalInput").ap()
        for l in range(3)
    ]
    identb_io = nc.dram_tensor("identB", [P, P], dt.float32, kind="ExternalInput").ap()
    identf8_io = nc.dram_tensor("identF8", [P, P], SEL_DT, kind="ExternalInput").ap()
    idx_ios = [
        nc.dram_tensor(f"idx{ch}", [P, S_ch[ch] // 16], dt.int16, kind="ExternalInput").ap()
        for ch in range(NCH)
    ]
    sel_io = nc.dram_tensor("sel", [P, NB_total * P], SEL_DT, kind="ExternalInput").ap()
    out_io = nc.dram_tensor("out3", [NPC, 32], dt.float32, kind="ExternalOutput").ap()

    with tile.TileContext(nc) as tc:
        with (
            tc.tile_pool(name="const", bufs=1) as constp,
            tc.tile_pool(name="hskeep", bufs=1) as hkp,
            tc.tile_pool(name="xT", bufs=8) as xTp,
            tc.tile_pool(name="idxw", bufs=8) as idxp,
            tc.tile_pool(name="msgs", bufs=MSG_BUFS) as msgp,
            tc.tile_pool(name="sel", bufs=SEL_BUFS) as selp,
            tc.tile_pool(name="outt", bufs=4) as outtp,
            tc.tile_pool(name="att", bufs=4) as attp,
            tc.tile_pool(name="psum_a", bufs=4, space="PSUM") as psap,
            tc.tile_pool(name="psum_t", bufs=2, space="PSUM") as pstp,
            tc.tile_pool(name="psum_d", bufs=2, space="PSUM") as psdp,
            tc.tile_pool(name="dram", bufs=1, space="DRAM") as dramp,
        ):
            # ---- constants ----
            dinv_sb = constp.tile([P, NTILES], dt.float32)
            nc.sync.dma_start(dinv_sb[:], dinv_io[:])
            rdinv_sb = constp.tile([1, NTILES * P], dt.bfloat16)
            nc.sync.dma_start(rdinv_sb[:], rdinv_io[:])
            w1_sb = constp.tile([P, F0], dt.bfloat16)
            nc.sync.dma_start(w1_sb[:], w1_io[:])
            w2_sb = constp.tile([P, 64], dt.bfloat16)
            nc.sync.dma_start(w2_sb[:], w2_io[:])
            w3_sb = constp.tile([64, 32], dt.bfloat16)
            nc.sync.dma_start(w3_sb[:], w3_io[:])
            brow_sb = []
            for l in range(3):
                bt = constp.tile([1, NCOLS[l]], dt.bfloat16, tag=f"brow{l}")
                nc.sync.dma_start(bt[:], brow_ios[l][:])
                brow_sb.append(bt)
            identb = constp.tile([P, P], dt.float32)
            nc.sync.dma_start(identb[:], identb_io[:])
            identf8 = constp.tile([P, P], SEL_DT)
            nc.sync.dma_start(identf8[:], identf8_io[:])

            hk = hkp.tile([P, NTILES * P], dt.bfloat16, tag="hk")

            bounces = [
                [
                    dramp.tile([QROWS[q], FW], dt.bfloat16, tag=f"bnc{l}_{q}",
                               name=f"bnc{l}_{q}")
                    for q in range(4)
                ]
                for l in range(3)
            ]
            tbls = [
                [
                    dramp.tile([CH_ROWS[q], FW], dt.bfloat16, tag=f"tbl{l}_{q}",
                               name=f"tbl{l}_{q}",
                               addr_space="Shared" if TBL_SHARED else "Local")
                    for q in range(4)
                ]
                for l in range(3)
            ]

            def emit_ag(l, q):
                nc.gpsimd.collective_compute(
                    "AllGather", mybir.AluOpType.bypass,
                    replica_groups=[list(range(N_CORES))],
                    ins=[bounces[l][q].opt()], outs=[tbls[l][q].opt()],
                )

            # ---- dense0: hs1 = dinv * (x @ W1) ----
            for t in range(NTILES):
                r = _rows(t)
                c0 = t * P
                q = _quarter_of_tile(t)
                pd = psdp.tile([P, P], dt.float32, space="PSUM", tag="pd")
                nk = F0 // P
                for k in range(nk):
                    xt = xTp.tile([P, P], dt.bfloat16, tag="xt")
                    nc.sync.dma_start(xt[:, :r], xT_io[k * P:(k + 1) * P, c0:c0 + r])
                    nc.tensor.matmul(
                        out=pd[:r, :], lhsT=xt[:, :r],
                        rhs=w1_sb[:, k * P:(k + 1) * P],
                        start=(k == 0), stop=(k == nk - 1),
                    )
                nc.vector.tensor_scalar_mul(
                    hk[:r, c0:c0 + P], pd[:r, :], dinv_sb[:r, t:t + 1]
                )
                nc.sync.dma_start(
                    bounces[0][q][c0 - QSTART[q]:c0 - QSTART[q] + r, :],
                    hk[:r, c0:c0 + P],
                )
                if t == QT[q + 1] - 1:
                    emit_ag(0, q)

            # ---- fused aggregation + next dense, per layer ----
            for l in range(3):
                ncols = NCOLS[l]
                fout = NCOLS[l + 1] if l < 2 else 0
                w_next = (w2_sb, w3_sb)[l] if l < 2 else None
                prev_q = -1
                for wi, (q, wtiles) in enumerate(waves):
                    first_of_q = prev_q != q
                    prev_q = q
                    nbs = wave_nb[wi]
                    gw0 = g_start[(wtiles[0], 0)]
                    nbw = wave_nbtot[wi]

                    selt = selp.tile([P, SEL_NB, P], SEL_DT, tag="sel")
                    nc.sync.dma_start(
                        selt[:, :nbw, :], sel_io[:, gw0 * P:(gw0 + nbw) * P]
                    )
                    msg_tiles = {}
                    for ch in range(NCH):
                        nb = nbs[ch]
                        mt = msgp.tile([P, MSG_NB, FW], dt.bfloat16, tag="msg")
                        msg_tiles[ch] = mt
                        if nb == 0:
                            continue
                        s0 = chunk_start[ch][wtiles[0]]
                        iw = idxp.tile([P, MSG_NB * 8], dt.int16, tag="idx")
                        nc.sync.dma_start(
                            iw[:, :nb * 8], idx_ios[ch][:, s0 // 16:(s0 + nb * P) // 16]
                        )
                        for b0 in range(0, nb, MAXB):
                            b1 = min(b0 + MAXB, nb)
                            Ssub = (b1 - b0) * P
                            nc.gpsimd.dma_gather(
                                out_ap=mt[:, b0:b1, :],
                                in_ap=tbls[l][ch][:, :],
                                idxs_ap=iw[:, b0 * 8:b1 * 8],
                                num_idxs=Ssub, num_idxs_reg=Ssub,
                                elem_size=FW, elem_step=FW,
                                single_packet=False,
                                queue_num=ch,
                            )
                    # pipelined quarter AllGather for the next layer's table
                    if first_of_q and q > 0 and l < 2:
                        emit_ag(l + 1, q - 1)

                    for t in wtiles:
                        r = _rows(t)
                        c0 = t * P
                        tq = _quarter_of_tile(t)
                        pa = psap.tile([P, ncols], dt.float32, space="PSUM", tag="pa")
                        # self-loop rows from the local hs tile (PSUM init)
                        nc.tensor.matmul(
                            out=pa[:r, :], lhsT=identf8[:r, :r],
                            rhs=hk[:r, c0:c0 + ncols],
                            start=True, stop=False,
                        )
                        mms = []
                        for ch in range(NCH):
                            coff = int(sum(blocks[tt, ch] for tt in wtiles[:wtiles.index(t)]))
                            for b in range(int(blocks[t, ch])):
                                mms.append((g_start[(t, ch)] + b - gw0, ch, coff + b))
                        # bias outer product: (1/dinv)[d] * b[f]; closes the
                        # accumulation group when a tile has no edge blocks
                        nc.tensor.matmul(
                            out=pa[:r, :], lhsT=rdinv_sb[0:1, c0:c0 + r],
                            rhs=brow_sb[l][0:1, :],
                            start=False, stop=(len(mms) == 0),
                        )
                        for i, (j, ch, col) in enumerate(mms):
                            nc.tensor.matmul(
                                out=pa[:r, :], lhsT=selt[:, j, :r],
                                rhs=msg_tiles[ch][:, col, :ncols],
                                start=False, stop=(i == len(mms) - 1),
                            )
                        if l < 2:
                            outt = outtp.tile([P, P], dt.float32, tag="outt")
                            nc.scalar.activation(
                                outt[:r, :ncols], pa[:r, :],
                                mybir.ActivationFunctionType.Relu,
                                scale=dinv_sb[:r, t:t + 1],
                            )
                            pst = pstp.tile([P, P], dt.float32, space="PSUM", tag="pt")
                            nc.tensor.transpose(
                                out=pst[:ncols, :r], in_=outt[:r, :ncols],
                                identity=identb[:r, :r],
                            )
                            att = attp.tile([P, P], dt.bfloat16, tag="att")
                            nc.scalar.copy(att[:ncols, :r], pst[:ncols, :r])
                            pd = psdp.tile([P, P], dt.float32, space="PSUM", tag="pd")
                            nc.tensor.matmul(
                                out=pd[:r, :fout], lhsT=att[:ncols, :r], rhs=w_next[:, :],
                                start=True, stop=True,
                            )
                            nc.vector.tensor_scalar_mul(
                                hk[:r, c0:c0 + fout], pd[:r, :fout], dinv_sb[:r, t:t + 1]
                            )
                            nc.sync.dma_start(
                                bounces[l + 1][tq][c0 - QSTART[tq]:c0 - QSTART[tq] + r, :],
                                hk[:r, c0:c0 + P],
                            )
                        else:
                            o32 = outtp.tile([P, 32], dt.float32, tag="o32")
                            nc.scalar.activation(
                                o32[:r, :], pa[:r, :32],
                                mybir.ActivationFunctionType.Relu,
                                scale=dinv_sb[:r, t:t + 1],
                            )
                            nc.sync.dma_start(out_io[c0:c0 + r, :], o32[:r, :])
                if l < 2:
                    emit_ag(l + 1, 3)

    nc.compile()
    return nc


def _pack_inputs(x, dinv, W1, b1, W2, b2, W3, b3, idx16, sel):
    w1sb = np.zeros((P, F0), np.float32)
    for k in range(F0 // P):
        w1sb[:, k * P:(k + 1) * P] = W1[k * P:(k + 1) * P, :]
    identb = np.eye(P, dtype=np.float32)

    in_maps = []
    for c in range(N_CORES):
        lo = c * NPC
        xs = x[lo:lo + NPC].astype(np.float32)
        dv = dinv[lo:lo + NPC]
        dvt = np.ones((P, NTILES), np.float32)
        rdv = np.zeros((1, NTILES * P), np.float32)
        for t in range(NTILES):
            r = _rows(t)
            dvt[:r, t] = dv[t * P:t * P + r]
            rdv[0, t * P:t * P + r] = 1.0 / dv[t * P:t * P + r]
        in_maps.append({
            "xT": np.ascontiguousarray(xs.T).astype(BF),
            "dinvT": dvt,
            "rdinvR": rdv.astype(BF),
            "W1sb": w1sb.astype(BF),
            "W2sb": W2.astype(BF),
            "W3sb": W3.astype(BF),
            "b1row": b1[None, :].astype(BF),
            "b2row": b2[None, :].astype(BF),
            "b3row": b3[None, :].astype(BF),
            "identB": identb.astype(np.float32),
            "identF8": identb.astype(SEL_NP),
            **{f"idx{ch}": idx16[ch][c] for ch in range(NCH)},
            "sel": sel[c].reshape(P, -1),
        })
    return in_maps


_TRACE = [False]          # set by test harness to request a profiled run
_LAST_RESULT = [None]     # BassKernelResults of the last run (for profiling)


def kernel(x, edge_index, batch, W1, b1, W2, b2, W3, b3, Wfc, bfc):
    x = np.asarray(x)
    edge_index = np.asarray(edge_index)
    batch = np.asarray(batch)
    W1, b1 = np.asarray(W1), np.asarray(b1)
    W2, b2 = np.asarray(W2), np.asarray(b2)
    W3, b3 = np.asarray(W3), np.asarray(b3)
    Wfc, bfc = np.asarray(Wfc), np.asarray(bfc)

    dinv, idx16, sel, meta = _host_prep(edge_index.astype(np.int64))
    nc = _build_program(meta)
    in_maps = _pack_inputs(x, dinv, W1, b1, W2, b2, W3, b3, idx16, sel)
    res = run_bass_kernel_spmd(
        nc, in_maps, core_ids=list(range(N_CORES)), trace=_TRACE[0]
    )
    _LAST_RESULT[0] = res

    h3 = np.concatenate([res.results[c]["out3"] for c in range(N_CORES)], axis=0)

    # host epilogue: segment max pool + FC + log_softmax (float64 for stability)
    pooled = np.full((N_GRAPHS, 32), -np.inf, np.float64)
    bnd = np.searchsorted(batch, np.arange(N_GRAPHS + 1))
    for g in range(N_GRAPHS):
        if bnd[g + 1] > bnd[g]:
            pooled[g] = h3[bnd[g]:bnd[g + 1]].max(axis=0)
    logits = pooled @ Wfc.astype(np.float64) + bfc.astype(np.float64)
    m = logits.max(axis=1, keepdims=True)
    lse = m + np.log(np.exp(logits - m).sum(axis=1, keepdims=True))
    return (logits - lse).astype(np.float32)
